# revision 3
# baseline (speedup 1.0000x reference)
"""Trainium2 Bass kernel for 12-head cross-attention with additive bias.

Reference computation (b=2, n=m=2048, e=768, h=12, d=64), all fp32:
    q  = x @ Wq.T;  kv = context @ Wkv.T;  k, v = split(kv)
    sim  = (q_h @ k_h.T) * d**-0.5 + attn_bias
    out_h = softmax(sim) @ v_h
    y = concat_heads(out) @ Wout.T + b_out

Sharding: 8 cores = 2 batches x 4 head-groups (3 heads each).  Each core
computes the projections for its head group, attention for its 3 heads, and
a partial output projection y_part[b] = out_g @ Wout[:, cols_g].T.  The host
sums the 4 per-group partials of each batch and adds b_out.

On-chip dataflow is fully "transposed" so no on-chip transposes are needed:
the host passes x.T / context.T / per-head-transposed bias, and weights as
the lhsT layouts the tensor engine wants (all cast to bf16; the softmax
scale is folded into Wq on the host):
    QT[d,q] / KT[d,m] = WT-chunks.T @ xT-chunks          (PE)
    ST[m,q]  = KT-slice.T @ QT  (+ bias via identity-matmul PSUM accum)
    PT       = exp(ST)                                    (ScalarE, PSUM->SBUF)
    OT[d,q] += V_aug-slice.T @ PT   (V_aug has a ones column -> denominators)
    y[q,j]  += OT-chunks.T @ WoutT-chunks   (normalized by 1/denom first)
"""

import numpy as np
import ml_dtypes

import concourse.bacc as bacc
import concourse.mybir as mybir
import concourse.tile as tile
from concourse.bass_utils import run_bass_kernel_spmd

BF16 = ml_dtypes.bfloat16

B, N, M, E = 2, 2048, 2048, 768
HEADS = 12
D = 64                      # head dim
GROUPS = 4                  # head groups (cores per batch)
HG = HEADS // GROUPS        # heads per group = 3
CG = HG * D                 # channels per group = 192
NCORES = 8

P = 128                     # partitions
QC = 512                    # q free-dim chunk
NQ = N // QC                # 4 q-chunks
MT = M // P                 # 16 m-tiles
EC = E // P                 # 6 contraction chunks
JC = 384                    # output-proj free chunk
NJ = E // JC                # 2 output-proj chunks

_CACHED_NC = None


def build_nc():
    f32 = mybir.dt.float32
    bf16 = mybir.dt.bfloat16

    nc = bacc.Bacc("TRN2", debug=False)
    xT = nc.dram_tensor("xT", [E, N], bf16, kind="ExternalInput")
    cT = nc.dram_tensor("cT", [E, M], bf16, kind="ExternalInput")
    biasT = nc.dram_tensor("biasT", [HG, M, N], bf16, kind="ExternalInput")
    wqT = nc.dram_tensor("wqT", [E, CG], bf16, kind="ExternalInput")
    wkT = nc.dram_tensor("wkT", [E, CG], bf16, kind="ExternalInput")
    wvT = nc.dram_tensor("wvT", [E, CG], bf16, kind="ExternalInput")
    woT = nc.dram_tensor("woT", [CG, E], bf16, kind="ExternalInput")
    ident = nc.dram_tensor("ident", [P, P], bf16, kind="ExternalInput")
    y = nc.dram_tensor("y", [N, E], f32, kind="ExternalOutput")

    with tile.TileContext(nc) as tc:
        with (
            tc.tile_pool(name="big", bufs=1) as big,
            tc.tile_pool(name="bias_pool", bufs=4) as bias_pool,
            tc.tile_pool(name="pt_pool", bufs=4) as pt_pool,
            tc.tile_pool(name="small", bufs=2) as small,
            tc.tile_pool(name="ps_s", bufs=2, space="PSUM") as ps_s,
            tc.tile_pool(name="ps_o", bufs=2, space="PSUM") as ps_o,
            tc.tile_pool(name="ps_proj", bufs=2, space="PSUM") as ps_proj,
            tc.tile_pool(name="ps_y", bufs=2, space="PSUM") as ps_y,
        ):
            # ---- load inputs ----
            x_sb = big.tile([P, EC, N], bf16)
            c_sb = big.tile([P, EC, M], bf16)
            nc.sync.dma_start(out=x_sb[:], in_=xT.rearrange("(c p) q -> p c q", p=P))
            nc.sync.dma_start(out=c_sb[:], in_=cT.rearrange("(c p) q -> p c q", p=P))

            wq_sb = big.tile([P, EC, CG], bf16)
            wk_sb = big.tile([P, EC, CG], bf16)
            wv_sb = big.tile([P, EC, CG], bf16)
            nc.sync.dma_start(out=wq_sb[:], in_=wqT.rearrange("(c p) d -> p c d", p=P))
            nc.sync.dma_start(out=wk_sb[:], in_=wkT.rearrange("(c p) d -> p c d", p=P))
            nc.sync.dma_start(out=wv_sb[:], in_=wvT.rearrange("(c p) d -> p c d", p=P))

            wo_sb = big.tile([P, 2, E], bf16)
            nc.sync.dma_start(out=wo_sb[:, 0, :], in_=woT[0:P, :])
            nc.sync.dma_start(out=wo_sb[0 : CG - P, 1, :], in_=woT[P:CG, :])

            id_sb = big.tile([P, P], bf16)
            nc.sync.dma_start(out=id_sb[:], in_=ident[:])

            # ---- projections ----
            # QT / KT: [d-on-partition, seq-free]; heads 0,1 share a
            # 128-partition tile, head 2 gets a 64-partition tile.
            qt01 = big.tile([P, N], bf16)
            qt2 = big.tile([D, N], bf16)
            kt01 = big.tile([P, M], bf16)
            kt2 = big.tile([D, M], bf16)

            for dst01, dst2, w_sb, src in (
                (qt01, qt2, wq_sb, x_sb),
                (kt01, kt2, wk_sb, c_sb),
            ):
                for jq in range(NQ):
                    pr01 = ps_proj.tile([P, QC], mybir.dt.float32, tag="proj")
                    for e in range(EC):
                        nc.tensor.matmul(
                            pr01[:],
                            w_sb[:, e, 0:P],
                            src[:, e, jq * QC : (jq + 1) * QC],
                            start=(e == 0),
                            stop=(e == EC - 1),
                        )
                    nc.scalar.copy(dst01[:, jq * QC : (jq + 1) * QC], pr01[:])
                    pr2 = ps_proj.tile([D, QC], mybir.dt.float32, tag="proj")
                    for e in range(EC):
                        nc.tensor.matmul(
                            pr2[:],
                            w_sb[:, e, P:CG],
                            src[:, e, jq * QC : (jq + 1) * QC],
                            start=(e == 0),
                            stop=(e == EC - 1),
                        )
                    nc.scalar.copy(dst2[:, jq * QC : (jq + 1) * QC], pr2[:])

            # V with an appended ones column (-> softmax denominators).
            v_sb = big.tile([P, MT, HG, D + 1], bf16)
            nc.vector.memset(v_sb[:, :, :, D], 1.0)
            for mt in range(MT):
                pv = ps_proj.tile([P, CG], mybir.dt.float32, tag="proj")
                for e in range(EC):
                    nc.tensor.matmul(
                        pv[:],
                        c_sb[:, e, mt * P : (mt + 1) * P],
                        wv_sb[:, e, :],
                        start=(e == 0),
                        stop=(e == EC - 1),
                    )
                nc.scalar.copy(
                    v_sb[:, mt, :, 0:D], pv.rearrange("p (h d) -> p h d", d=D)
                )

            # ---- attention + output projection ----
            ot01 = big.tile([P, N], bf16)
            ot2 = big.tile([D, N], bf16)

            for jq in range(NQ):
                qs = slice(jq * QC, (jq + 1) * QC)
                for h in range(HG):
                    if h < 2:
                        kt_h = (kt01, h * D)
                        qt_h = (qt01, h * D)
                    else:
                        kt_h = (kt2, 0)
                        qt_h = (qt2, 0)
                    kt_t, kt_o = kt_h
                    qt_t, qt_o = qt_h

                    o_ps = ps_o.tile([D + 1, QC], mybir.dt.float32, tag="ops")
                    for mtb in range(0, MT, 4):
                        # one DMA covers 4 m-tiles of bias
                        bias_sb = bias_pool.tile([P, 4, QC], bf16, tag="bias")
                        nc.sync.dma_start(
                            out=bias_sb[:],
                            in_=biasT[
                                h, mtb * P : (mtb + 4) * P, qs
                            ].rearrange("(t p) q -> p t q", p=P),
                        )
                        for ti in range(4):
                            mt = mtb + ti
                            s_ps = ps_s.tile([P, QC], mybir.dt.float32, tag="spsum")
                            nc.tensor.matmul(
                                s_ps[:],
                                kt_t[kt_o : kt_o + D, mt * P : (mt + 1) * P],
                                qt_t[qt_o : qt_o + D, qs],
                                start=True,
                                stop=False,
                            )
                            nc.tensor.matmul(
                                s_ps[:],
                                id_sb[:],
                                bias_sb[:, ti, :],
                                start=False,
                                stop=True,
                            )
                            pt = pt_pool.tile([P, QC], bf16, tag="pt")
                            nc.scalar.activation(
                                pt[:], s_ps[:], mybir.ActivationFunctionType.Exp
                            )
                            nc.tensor.matmul(
                                o_ps[:],
                                v_sb[:, mt, h, :],
                                pt[:],
                                start=(mt == 0),
                                stop=(mt == MT - 1),
                            )
                    recip = small.tile([1, QC], mybir.dt.float32, tag="recip")
                    nc.vector.reciprocal(recip[:], o_ps[D : D + 1, :])
                    recip_bc = small.tile([D, QC], mybir.dt.float32, tag="recipbc")
                    nc.gpsimd.partition_broadcast(recip_bc[:], recip[:])
                    if h < 2:
                        dst = ot01[h * D : (h + 1) * D, qs]
                    else:
                        dst = ot2[:, qs]
                    nc.vector.tensor_mul(dst, o_ps[0:D, :], recip_bc[:])

                # output projection for this q-chunk (4 q-tiles of 128 rows)
                for qt in range(jq * NQ, (jq + 1) * NQ):
                    qsl = slice(qt * P, (qt + 1) * P)
                    for jn in range(NJ):
                        jsl = slice(jn * JC, (jn + 1) * JC)
                        y_ps = ps_y.tile([P, JC], mybir.dt.float32, tag="ypsum")
                        nc.tensor.matmul(
                            y_ps[:], ot01[:, qsl], wo_sb[:, 0, jsl],
                            start=True, stop=False,
                        )
                        nc.tensor.matmul(
                            y_ps[:],
                            ot2[:, qsl],
                            wo_sb[0 : CG - P, 1, jsl],
                            start=False,
                            stop=True,
                        )
                        y_sb = small.tile([P, JC], mybir.dt.float32, tag="ysb")
                        nc.scalar.copy(y_sb[:], y_ps[:])
                        nc.sync.dma_start(out=y[qsl, jsl], in_=y_sb[:])

    nc.compile()
    return nc


def _shard_inputs(x, context, attn_bias, Wq, Wkv, Wout):
    scale = D ** -0.5
    ident = np.eye(P, dtype=BF16)
    in_maps = []
    for core in range(NCORES):
        b, g = divmod(core, GROUPS)
        cs = slice(g * CG, (g + 1) * CG)
        in_maps.append(
            {
                "xT": np.ascontiguousarray(x[b].T).astype(BF16),
                "cT": np.ascontiguousarray(context[b].T).astype(BF16),
                "biasT": np.ascontiguousarray(
                    attn_bias[b, g * HG : (g + 1) * HG].transpose(0, 2, 1)
                ).astype(BF16),
                "wqT": (Wq[cs, :].T * scale).astype(BF16),
                "wkT": np.ascontiguousarray(Wkv[cs, :].T).astype(BF16),
                "wvT": np.ascontiguousarray(Wkv[E + cs.start : E + cs.stop, :].T).astype(BF16),
                "woT": np.ascontiguousarray(Wout[:, cs].T).astype(BF16),
                "ident": ident,
            }
        )
    return in_maps


def kernel(x, context, attn_bias, Wq, Wkv, Wout, b_out):
    global _CACHED_NC
    if _CACHED_NC is None:
        _CACHED_NC = build_nc()
    nc = _CACHED_NC

    in_maps = _shard_inputs(x, context, attn_bias, Wq, Wkv, Wout)
    res = run_bass_kernel_spmd(nc, in_maps, list(range(NCORES)))

    out = np.zeros((B, N, E), dtype=np.float32)
    for core in range(NCORES):
        out[core // GROUPS] += res.results[core]["y"]
    out += b_out.astype(np.float32)
    return out


# revision 15
# speedup vs baseline: 1.0043x; 1.0043x over previous
"""Trainium2 Bass kernel for 12-head cross-attention with additive bias.

Reference computation (b=2, n=m=2048, e=768, h=12, d=64), all fp32:
    q  = x @ Wq.T;  kv = context @ Wkv.T;  k, v = split(kv)
    sim  = (q_h @ k_h.T) * d**-0.5 + attn_bias
    out_h = softmax(sim) @ v_h
    y = concat_heads(out) @ Wout.T + b_out

Sharding: 8 cores = 2 batches x 4 head-groups (3 heads each).  Each core
computes the projections for its head group, attention for its 3 heads, and
a partial output projection y_part[b] = out_g @ Wout[:, cols_g].T.  The host
sums the 4 per-group partials of each batch and adds b_out.

On-chip dataflow is fully "transposed" so no on-chip transposes are needed:
the host passes x.T / context.T / per-head-transposed bias, and weights as
the lhsT layouts the tensor engine wants (all cast to bf16; the softmax
scale is folded into Wq on the host):
    QT[d,q] / KT[d,m] = WT-chunks.T @ xT-chunks          (PE)
    ST[m,q]  = KT-slice.T @ QT   (h0/h1 row-packed on the PE array)
    ST      += bias     (identity-matmul on PE, or in-place DVE add -- split)
    PT       = exp(ST)                                    (ScalarE, PSUM->SBUF)
    OT[d,q] += V_aug-slice.T @ PT   (V_aug has a ones column -> denominators)
    y[q,j]  += OT-chunks.T @ WoutT-chunks   (normalized by 1/denom first)
"""

import numpy as np
import ml_dtypes

import concourse.bacc as bacc
import concourse.mybir as mybir
import concourse.tile as tile
from concourse.bass_utils import run_bass_kernel_spmd

BF16 = ml_dtypes.bfloat16

B, N, M, E = 2, 2048, 2048, 768
HEADS = 12
D = 64                      # head dim
GROUPS = 4                  # head groups (cores per batch)
HG = HEADS // GROUPS        # heads per group = 3
CG = HG * D                 # channels per group = 192
NCORES = 8

P = 128                     # partitions
QC = 512                    # q free-dim chunk
NQ = N // QC                # 4 q-chunks
MT = M // P                 # 16 m-tiles
MB = 4                      # m-tiles per bias DMA
EC = E // P                 # 6 contraction chunks
JC = 384                    # output-proj free chunk
NJ = E // JC                # 2 output-proj chunks

# bias-add engine split: if False, alternate PE identity-matmul / DVE add
PE_BIAS_ALL = False

_CACHED_NC = None


def build_nc(reps=1, sp_bufs=3, gen_bufs=2, pe_bias_all=None, pt_bufs=4, bias_bufs=6):
    f32 = mybir.dt.float32
    bf16 = mybir.dt.bfloat16

    if pe_bias_all is None:
        pe_bias_all = PE_BIAS_ALL
    nc = bacc.Bacc("TRN2", debug=False)
    xT = nc.dram_tensor("xT", [E, N], bf16, kind="ExternalInput")
    cT = nc.dram_tensor("cT", [E, M], bf16, kind="ExternalInput")
    biasT = nc.dram_tensor("biasT", [HG, M, N], bf16, kind="ExternalInput")
    wqT = nc.dram_tensor("wqT", [E, CG], bf16, kind="ExternalInput")
    wkT = nc.dram_tensor("wkT", [E, CG], bf16, kind="ExternalInput")
    wvT = nc.dram_tensor("wvT", [E, CG], bf16, kind="ExternalInput")
    woT = nc.dram_tensor("woT", [CG, E], bf16, kind="ExternalInput")
    ident = nc.dram_tensor("ident", [P, P], bf16, kind="ExternalInput")
    y = nc.dram_tensor("y", [N, E], f32, kind="ExternalOutput")

    with tile.TileContext(nc) as tc:
        with (
            tc.tile_pool(name="big", bufs=1) as big,
            tc.tile_pool(name="bias_pool", bufs=bias_bufs) as bias_pool,
            tc.tile_pool(name="pt_pool", bufs=pt_bufs) as pt_pool,
            tc.tile_pool(name="small", bufs=2) as small,
            tc.tile_pool(name="ysb_pool", bufs=3) as ysb_pool,
            tc.tile_pool(name="ps_sp", bufs=sp_bufs, space="PSUM") as ps_sp,
            tc.tile_pool(name="ps_gen", bufs=gen_bufs, space="PSUM") as ps_gen,
        ):
          for _rep in range(reps):
            # ---- load inputs (context first: KT/V unblock the S matmuls) ----
            x_sb = big.tile([P, EC, N], bf16)
            c_sb = big.tile([P, EC, M], bf16)
            for e in range(EC):
                nc.sync.dma_start(out=c_sb[:, e, :], in_=cT[e * P : (e + 1) * P, :])

            wq_sb = big.tile([P, EC, CG], bf16)
            wk_sb = big.tile([P, EC, CG], bf16)
            wv_sb = big.tile([P, EC, CG], bf16)
            nc.sync.dma_start(out=wk_sb[:], in_=wkT.rearrange("(c p) d -> p c d", p=P))
            nc.sync.dma_start(out=wv_sb[:], in_=wvT.rearrange("(c p) d -> p c d", p=P))
            nc.sync.dma_start(out=wq_sb[:], in_=wqT.rearrange("(c p) d -> p c d", p=P))

            wo_sb = big.tile([P, 2, E], bf16)
            nc.sync.dma_start(out=wo_sb[:, 0, :], in_=woT[0:P, :])
            nc.sync.dma_start(out=wo_sb[0 : CG - P, 1, :], in_=woT[P:CG, :])

            id_sb = big.tile([P, P], bf16)
            nc.sync.dma_start(out=id_sb[:], in_=ident[:])

            for e in range(EC):
                nc.sync.dma_start(out=x_sb[:, e, :], in_=xT[e * P : (e + 1) * P, :])

            # ---- KT projection (replicated to both partition halves) ----
            kt = [big.tile([P, M], bf16, name=f"kt{_h}") for _h in range(HG)]
            for jq in range(NQ):
                qs = slice(jq * QC, (jq + 1) * QC)
                pr01 = ps_gen.tile([P, QC], f32, tag="gen")
                for e in range(EC):
                    nc.tensor.matmul(
                        pr01[:],
                        wk_sb[:, e, 0:P],
                        c_sb[:, e, qs],
                        start=(e == 0),
                        stop=(e == EC - 1),
                    )
                nc.vector.tensor_copy(kt[0][0:D, qs], pr01[0:D, :])
                nc.vector.tensor_copy(kt[1][0:D, qs], pr01[D:P, :])
                pr2 = ps_gen.tile([D, QC], f32, tag="gen")
                for e in range(EC):
                    nc.tensor.matmul(
                        pr2[:],
                        wk_sb[:, e, P:CG],
                        c_sb[:, e, qs],
                        start=(e == 0),
                        stop=(e == EC - 1),
                    )
                nc.vector.tensor_copy(kt[2][0:D, qs], pr2[:])
            for h in range(HG):
                nc.vector.tensor_copy(kt[h][D:P, :], kt[h][0:D, :])

            # ---- V projection helper (emitted per-mtb inside jq0/h0) ----
            v_sb = big.tile([P, MT, HG, D + 1], bf16)
            nc.vector.memset(v_sb[:, :, :, D], 1.0)

            def emit_vproj(mt):
                pv = ps_gen.tile([P, CG], f32, tag="gen", name="pv")
                for e in range(EC):
                    nc.tensor.matmul(
                        pv[:],
                        c_sb[:, e, mt * P : (mt + 1) * P],
                        wv_sb[:, e, :],
                        start=(e == 0),
                        stop=(e == EC - 1),
                    )
                nc.vector.tensor_copy(
                    v_sb[:, mt, :, 0:D], pv.rearrange("p (h d) -> p h d", d=D)
                )

            # ---- per q-chunk: QT projection, attention, output projection ----
            qt = [big.tile([P, N], bf16, name=f"qt{_h}") for _h in range(HG)]
            ot01 = big.tile([P, N], bf16)
            ot2 = big.tile([D, N], bf16)

            def emit_ygroup(qtile, jn, act_copy=False):
                qsl = slice(qtile * P, (qtile + 1) * P)
                jsl = slice(jn * JC, (jn + 1) * JC)
                y_ps = ps_gen.tile([P, JC], f32, tag="gen", name="y_ps")
                nc.tensor.matmul(
                    y_ps[:], ot01[:, qsl], wo_sb[:, 0, jsl],
                    start=True, stop=False,
                )
                nc.tensor.matmul(
                    y_ps[:], ot2[:, qsl], wo_sb[0 : CG - P, 1, jsl],
                    start=False, stop=True,
                )
                y_sb = ysb_pool.tile([P, JC], f32, tag="ysb", name="y_sb")
                if act_copy:
                    nc.scalar.copy(y_sb[:], y_ps[:])
                else:
                    nc.vector.tensor_copy(y_sb[:], y_ps[:])
                nc.gpsimd.dma_start(out=y[qsl, jsl], in_=y_sb[:])

            for jq in range(NQ):
                qs = slice(jq * QC, (jq + 1) * QC)
                # QT for this q-chunk, replicated to both halves
                pr01 = ps_gen.tile([P, QC], f32, tag="gen")
                for e in range(EC):
                    nc.tensor.matmul(
                        pr01[:], wq_sb[:, e, 0:P], x_sb[:, e, qs],
                        start=(e == 0), stop=(e == EC - 1),
                    )
                nc.vector.tensor_copy(qt[0][0:D, qs], pr01[0:D, :])
                nc.vector.tensor_copy(qt[1][0:D, qs], pr01[D:P, :])
                pr2 = ps_gen.tile([D, QC], f32, tag="gen")
                for e in range(EC):
                    nc.tensor.matmul(
                        pr2[:], wq_sb[:, e, P:CG], x_sb[:, e, qs],
                        start=(e == 0), stop=(e == EC - 1),
                    )
                nc.vector.tensor_copy(qt[2][0:D, qs], pr2[:])
                for h in range(HG):
                    nc.vector.tensor_copy(qt[h][D:P, qs], qt[h][0:D, qs])

                for h in range(HG):
                    o_ps = ps_gen.tile([D + 1, QC], f32, tag="gen", name="ops")
                    for mtb in range(0, MT, MB):
                        if jq == 0 and h == 0:
                            for _mt in range(mtb, mtb + MB):
                                emit_vproj(_mt)
                        bias_sb = bias_pool.tile([P, MB, QC], bf16, tag="bias")
                        nc.sync.dma_start(
                            out=bias_sb[:],
                            in_=biasT[
                                h, mtb * P : (mtb + MB) * P, qs
                            ].rearrange("(t p) q -> p t q", p=P),
                        )
                        for tp in range(0, MB, 2):
                            # two m-tiles share one 2-bank PSUM tile; their S
                            # matmuls use disjoint PE row groups and run packed
                            sp = ps_sp.tile([P, 2 * QC], f32, tag="sp")
                            for half in range(2):
                                mt = mtb + tp + half
                                ro = half * D
                                use_dve = (not pe_bias_all) and (mt + h) % 2 == 0
                                nc.tensor.matmul(
                                    sp[:, half * QC : (half + 1) * QC],
                                    kt[h][ro : ro + D, mt * P : (mt + 1) * P],
                                    qt[h][ro : ro + D, qs],
                                    start=True,
                                    stop=use_dve,
                                )
                            for half in range(2):
                                mt = mtb + tp + half
                                ti = tp + half
                                dst = sp[:, half * QC : (half + 1) * QC]
                                if (not pe_bias_all) and (mt + h) % 2 == 0:
                                    nc.vector.tensor_add(
                                        dst, dst, bias_sb[:, ti, :]
                                    )
                                else:
                                    nc.tensor.matmul(
                                        dst,
                                        id_sb[:],
                                        bias_sb[:, ti, :],
                                        start=False,
                                        stop=True,
                                    )
                            pt = pt_pool.tile([P, 2 * QC], bf16, tag="pt")
                            nc.scalar.activation(
                                pt[:], sp[:], mybir.ActivationFunctionType.Exp
                            )
                            for half in range(2):
                                mt = mtb + tp + half
                                nc.tensor.matmul(
                                    o_ps[:],
                                    v_sb[:, mt, h, :],
                                    pt[:, half * QC : (half + 1) * QC],
                                    start=(mt == 0),
                                    stop=(mt == MT - 1),
                                )
                    recip = small.tile([1, QC], f32, tag="recip")
                    nc.vector.reciprocal(recip[:], o_ps[D : D + 1, :])
                    recip_bc = small.tile([D, QC], f32, tag="recipbc")
                    nc.gpsimd.partition_broadcast(recip_bc[:], recip[:])
                    if h < 2:
                        dst = ot01[h * D : (h + 1) * D, qs]
                    else:
                        dst = ot2[:, qs]
                    nc.vector.tensor_mul(dst, o_ps[0:D, :], recip_bc[:])
                    if jq > 0:
                        pj = jq - 1
                        groups = [(qq, jn) for qq in range(pj * NQ, (pj + 1) * NQ)
                                  for jn in range(NJ)]
                        lo = (len(groups) * h) // HG
                        hi = (len(groups) * (h + 1)) // HG
                        for qq, jn in groups[lo:hi]:
                            emit_ygroup(qq, jn)


            for qq in range((NQ - 1) * NQ, NQ * NQ):
                for jn in range(NJ):
                    emit_ygroup(qq, jn, act_copy=True)

    nc.compile()
    return nc


def _shard_inputs(x, context, attn_bias, Wq, Wkv, Wout):
    scale = D ** -0.5
    ident = np.eye(P, dtype=BF16)
    in_maps = []
    for core in range(NCORES):
        b, g = divmod(core, GROUPS)
        cs = slice(g * CG, (g + 1) * CG)
        in_maps.append(
            {
                "xT": np.ascontiguousarray(x[b].T).astype(BF16),
                "cT": np.ascontiguousarray(context[b].T).astype(BF16),
                "biasT": np.ascontiguousarray(
                    attn_bias[b, g * HG : (g + 1) * HG].transpose(0, 2, 1)
                ).astype(BF16),
                "wqT": (Wq[cs, :].T * scale).astype(BF16),
                "wkT": np.ascontiguousarray(Wkv[cs, :].T).astype(BF16),
                "wvT": np.ascontiguousarray(Wkv[E + cs.start : E + cs.stop, :].T).astype(BF16),
                "woT": np.ascontiguousarray(Wout[:, cs].T).astype(BF16),
                "ident": ident,
            }
        )
    return in_maps


def kernel(x, context, attn_bias, Wq, Wkv, Wout, b_out):
    global _CACHED_NC
    if _CACHED_NC is None:
        _CACHED_NC = build_nc()
    nc = _CACHED_NC

    in_maps = _shard_inputs(x, context, attn_bias, Wq, Wkv, Wout)
    res = run_bass_kernel_spmd(nc, in_maps, list(range(NCORES)))

    out = np.zeros((B, N, E), dtype=np.float32)
    for core in range(NCORES):
        out[core // GROUPS] += res.results[core]["y"]
    out += b_out.astype(np.float32)
    return out


# revision 26
# speedup vs baseline: 74.2442x; 73.9233x over previous
"""Trainium2 Bass kernel for 12-head cross-attention with additive bias.

Reference computation (b=2, n=m=2048, e=768, h=12, d=64), all fp32:
    q  = x @ Wq.T;  kv = context @ Wkv.T;  k, v = split(kv)
    sim  = (q_h @ k_h.T) * d**-0.5 + attn_bias
    out_h = softmax(sim) @ v_h
    y = concat_heads(out) @ Wout.T + b_out

Sharding: 8 cores = 2 batches x 4 head-groups (3 heads each).  Each core
computes the projections for its head group, attention for its 3 heads, and
a partial output projection y_part[b] = out_g @ Wout[:, cols_g].T.  The host
sums the 4 per-group partials of each batch and adds b_out.

On-chip dataflow is fully "transposed" so no on-chip transposes are needed:
the host passes x.T / context.T / per-head-transposed bias, and weights as
the lhsT layouts the tensor engine wants (all cast to bf16; the softmax
scale is folded into Wq on the host):
    QT[d,q] / KT[d,m] = WT-chunks.T @ xT-chunks          (PE)
    ST[m,q]  = KT-slice.T @ QT   (h0/h1 row-packed on the PE array)
    ST      += bias     (identity-matmul on PE, or in-place DVE add -- split)
    PT       = exp(ST)                                    (ScalarE, PSUM->SBUF)
    OT[d,q] += V_aug-slice.T @ PT   (V_aug has a ones column -> denominators)
    y[q,j]  += OT-chunks.T @ WoutT-chunks   (normalized by 1/denom first)
"""

import numpy as np
import ml_dtypes

import concourse.bacc as bacc
import concourse.mybir as mybir
import concourse.tile as tile
from concourse.bass_utils import run_bass_kernel_spmd

BF16 = ml_dtypes.bfloat16

B, N, M, E = 2, 2048, 2048, 768
HEADS = 12
D = 64                      # head dim
GROUPS = 4                  # head groups (cores per batch)
HG = HEADS // GROUPS        # heads per group = 3
CG = HG * D                 # channels per group = 192
NCORES = 8

P = 128                     # partitions
QC = 512                    # q free-dim chunk
NQ = N // QC                # 4 q-chunks
MT = M // P                 # 16 m-tiles
MB = 4                      # m-tiles per bias DMA
EC = E // P                 # 6 contraction chunks
JC = 384                    # output-proj free chunk
NJ = E // JC                # 2 output-proj chunks

# bias-add engine split: of every 8 (mt,h) tiles, this many go to the DVE
PE_BIAS_ALL = False
DVE_FRAC8 = 0

_CACHED_NC = None


def build_nc(reps=1, sp_bufs=3, gen_bufs=2, pe_bias_all=None, pt_bufs=6, bias_bufs=6, split_exp=False, dve_frac=None):
    f32 = mybir.dt.float32
    bf16 = mybir.dt.bfloat16

    if pe_bias_all is None:
        pe_bias_all = PE_BIAS_ALL
    if dve_frac is None:
        dve_frac = DVE_FRAC8
    nc = bacc.Bacc("TRN2", debug=False)
    xT = nc.dram_tensor("xT", [E, N], bf16, kind="ExternalInput")
    cT = nc.dram_tensor("cT", [E, M], bf16, kind="ExternalInput")
    biasT = nc.dram_tensor("biasT", [HG, M, N], bf16, kind="ExternalInput")
    wqT = nc.dram_tensor("wqT", [E, CG], bf16, kind="ExternalInput")
    wkT = nc.dram_tensor("wkT", [E, CG], bf16, kind="ExternalInput")
    wvT = nc.dram_tensor("wvT", [E, CG], bf16, kind="ExternalInput")
    woT = nc.dram_tensor("woT", [CG, E], bf16, kind="ExternalInput")
    ident = nc.dram_tensor("ident", [P, P], bf16, kind="ExternalInput")
    y = nc.dram_tensor("y", [N, E], f32, kind="ExternalOutput")

    with tile.TileContext(nc) as tc:
        with (
            tc.tile_pool(name="big", bufs=1) as big,
            tc.tile_pool(name="bias_pool", bufs=bias_bufs) as bias_pool,
            tc.tile_pool(name="pt_pool", bufs=pt_bufs) as pt_pool,
            tc.tile_pool(name="small", bufs=2) as small,
            tc.tile_pool(name="ysb_pool", bufs=3) as ysb_pool,
            tc.tile_pool(name="ps_sp", bufs=sp_bufs, space="PSUM") as ps_sp,
            tc.tile_pool(name="ps_gen", bufs=gen_bufs, space="PSUM") as ps_gen,
        ):
          for _rep in range(reps):
            # ---- load inputs (context first: KT/V unblock the S matmuls) ----
            wq_sb = big.tile([P, EC, CG], bf16)
            wk_sb = big.tile([P, EC, CG], bf16)
            wv_sb = big.tile([P, EC, CG], bf16)
            wo_sb = big.tile([P, 2, E], bf16)
            id_sb = big.tile([P, P], bf16)
            c_sb = [big.tile([P, M], bf16, name=f"c{_e}") for _e in range(EC)]
            x_sb = [big.tile([P, N], bf16, name=f"x{_e}") for _e in range(EC)]

            nc.sync.dma_start(out=wk_sb[:], in_=wkT.rearrange("(c p) d -> p c d", p=P))
            for e in range(EC):
                nc.sync.dma_start(out=c_sb[e][:], in_=cT[e * P : (e + 1) * P, :])
            nc.sync.dma_start(out=wv_sb[:], in_=wvT.rearrange("(c p) d -> p c d", p=P))
            nc.sync.dma_start(out=wq_sb[:], in_=wqT.rearrange("(c p) d -> p c d", p=P))
            nc.sync.dma_start(out=wo_sb[:, 0, :], in_=woT[0:P, :])
            nc.sync.dma_start(out=wo_sb[0 : CG - P, 1, :], in_=woT[P:CG, :])
            nc.sync.dma_start(out=id_sb[:], in_=ident[:])
            for e in range(EC):
                nc.sync.dma_start(out=x_sb[e][:], in_=xT[e * P : (e + 1) * P, :])

            qt = [big.tile([P, N], bf16, name=f"qt{_h}") for _h in range(HG)]
            ot01 = big.tile([P, N], bf16)
            ot2 = big.tile([D, N], bf16)

            def emit_qtproj0(jq_):
                qs_ = slice(jq_ * QC, (jq_ + 1) * QC)
                pr01 = ps_gen.tile([P, QC], f32, tag="gen", name="pr01")
                for e in range(EC):
                    nc.tensor.matmul(
                        pr01[:], wq_sb[:, e, 0:P], x_sb[e][:, qs_],
                        start=(e == 0), stop=(e == EC - 1),
                    )
                nc.vector.tensor_copy(qt[0][0:D, qs_], pr01[0:D, :])
                nc.vector.tensor_copy(qt[1][0:D, qs_], pr01[D:P, :])
                pr2 = ps_gen.tile([D, QC], f32, tag="gen", name="pr2")
                for e in range(EC):
                    nc.tensor.matmul(
                        pr2[:], wq_sb[:, e, P:CG], x_sb[e][:, qs_],
                        start=(e == 0), stop=(e == EC - 1),
                    )
                nc.vector.tensor_copy(qt[2][0:D, qs_], pr2[:])
                for h_ in range(HG):
                    nc.vector.tensor_copy(qt[h_][D:P, qs_], qt[h_][0:D, qs_])

            # ---- KT projection (replicated to both partition halves) ----
            kt = [big.tile([P, M], bf16, name=f"kt{_h}") for _h in range(HG)]
            for jq in range(NQ):
                qs = slice(jq * QC, (jq + 1) * QC)
                pr01 = ps_gen.tile([P, QC], f32, tag="gen")
                for e in range(EC):
                    nc.tensor.matmul(
                        pr01[:],
                        wk_sb[:, e, 0:P],
                        c_sb[e][:, qs],
                        start=(e == 0),
                        stop=(e == EC - 1),
                    )
                nc.vector.tensor_copy(kt[0][0:D, qs], pr01[0:D, :])
                nc.vector.tensor_copy(kt[1][0:D, qs], pr01[D:P, :])
                pr2 = ps_gen.tile([D, QC], f32, tag="gen")
                for e in range(EC):
                    nc.tensor.matmul(
                        pr2[:],
                        wk_sb[:, e, P:CG],
                        c_sb[e][:, qs],
                        start=(e == 0),
                        stop=(e == EC - 1),
                    )
                nc.vector.tensor_copy(kt[2][0:D, qs], pr2[:])
                for h in range(HG):
                    nc.vector.tensor_copy(kt[h][D:P, qs], kt[h][0:D, qs])

            # ---- V projection helper (emitted per-mtb inside jq0/h0) ----
            v_sb = big.tile([P, MT, HG, D + 1], bf16)
            nc.vector.memset(v_sb[:, :, :, D], 1.0)

            def emit_vproj(mt):
                pv = ps_gen.tile([P, CG], f32, tag="gen", name="pv")
                for e in range(EC):
                    nc.tensor.matmul(
                        pv[:],
                        c_sb[e][:, mt * P : (mt + 1) * P],
                        wv_sb[:, e, :],
                        start=(e == 0),
                        stop=(e == EC - 1),
                    )
                nc.vector.tensor_copy(
                    v_sb[:, mt, :, 0:D], pv.rearrange("p (h d) -> p h d", d=D)
                )


            def emit_ygroup(qtile, act_copy=False):
                qsl = slice(qtile * P, (qtile + 1) * P)
                y_sb = ysb_pool.tile([P, E], f32, tag="ysb", name="y_sb")
                for jn in range(NJ):
                    jsl = slice(jn * JC, (jn + 1) * JC)
                    y_ps = ps_gen.tile([P, JC], f32, tag="gen", name="y_ps")
                    nc.tensor.matmul(
                        y_ps[:], ot01[:, qsl], wo_sb[:, 0, jsl],
                        start=True, stop=False,
                    )
                    nc.tensor.matmul(
                        y_ps[:], ot2[:, qsl], wo_sb[0 : CG - P, 1, jsl],
                        start=False, stop=True,
                    )
                    if act_copy:
                        nc.scalar.copy(y_sb[:, jsl], y_ps[:])
                    else:
                        nc.vector.tensor_copy(y_sb[:, jsl], y_ps[:])
                if act_copy:
                    nc.sync.dma_start(out=y[qsl, :], in_=y_sb[:])
                else:
                    nc.gpsimd.dma_start(out=y[qsl, :], in_=y_sb[:])

            emit_qtproj = emit_qtproj0
            emit_qtproj(0)
            for jq in range(NQ):
                qs = slice(jq * QC, (jq + 1) * QC)
                if jq > 0:
                    pj = jq - 1
                    ygroups = list(range(pj * NQ, (pj + 1) * NQ))
                else:
                    ygroups = []
                for h in range(HG):
                    o_ps = ps_gen.tile([D + 1, QC], f32, tag="gen", name="ops")
                    for mtb in range(0, MT, MB):
                        bias_sb = bias_pool.tile([P, MB, QC], bf16, tag="bias")
                        nc.sync.dma_start(
                            out=bias_sb[:],
                            in_=biasT[
                                h, mtb * P : (mtb + MB) * P, qs
                            ].rearrange("(t p) q -> p t q", p=P),
                        )
                        for tp in range(0, MB, 2):
                            # two m-tiles share one 2-bank PSUM tile; their S
                            # matmuls use disjoint PE row groups and run packed
                            sp = ps_sp.tile([P, 2 * QC], f32, tag="sp")
                            for half in range(2):
                                mt = mtb + tp + half
                                ro = half * D
                                use_dve = (not pe_bias_all) and (mt + h * 3) % 8 < dve_frac
                                nc.tensor.matmul(
                                    sp[:, half * QC : (half + 1) * QC],
                                    kt[h][ro : ro + D, mt * P : (mt + 1) * P],
                                    qt[h][ro : ro + D, qs],
                                    start=True,
                                    stop=use_dve,
                                )
                            for half in range(2):
                                mt = mtb + tp + half
                                ti = tp + half
                                dst = sp[:, half * QC : (half + 1) * QC]
                                if (not pe_bias_all) and (mt + h * 3) % 8 < dve_frac:
                                    nc.vector.tensor_add(
                                        dst, dst, bias_sb[:, ti, :]
                                    )
                                else:
                                    nc.tensor.matmul(
                                        dst,
                                        id_sb[:],
                                        bias_sb[:, ti, :],
                                        start=False,
                                        stop=True,
                                    )
                            pt = pt_pool.tile([P, 2 * QC], bf16, tag="pt")
                            if split_exp:
                                nc.scalar.activation(
                                    pt[:, 0:QC], sp[:, 0:QC],
                                    mybir.ActivationFunctionType.Exp,
                                )
                                nc.scalar.activation(
                                    pt[:, QC : 2 * QC], sp[:, QC : 2 * QC],
                                    mybir.ActivationFunctionType.Exp,
                                )
                            else:
                                nc.scalar.activation(
                                    pt[:], sp[:], mybir.ActivationFunctionType.Exp
                                )
                            if jq == 0 and h == 0:
                                emit_vproj(mtb + tp)
                                emit_vproj(mtb + tp + 1)
                            if ygroups and (tp // 2) % 2 == 1:
                                emit_ygroup(ygroups.pop(0))
                            for half in range(2):
                                mt = mtb + tp + half
                                nc.tensor.matmul(
                                    o_ps[:],
                                    v_sb[:, mt, h, :],
                                    pt[:, half * QC : (half + 1) * QC],
                                    start=(mt == 0),
                                    stop=(mt == MT - 1),
                                )
                    if h == 0 and jq + 1 < NQ:
                        emit_qtproj(jq + 1)
                    recip = small.tile([1, QC], f32, tag="recip")
                    nc.vector.reciprocal(recip[:], o_ps[D : D + 1, :])
                    recip_bc = small.tile([D, QC], f32, tag="recipbc")
                    nc.gpsimd.partition_broadcast(recip_bc[:], recip[:])
                    if h < 2:
                        dst = ot01[h * D : (h + 1) * D, qs]
                    else:
                        dst = ot2[:, qs]
                    nc.vector.tensor_mul(dst, o_ps[0:D, :], recip_bc[:])
                while ygroups:
                    emit_ygroup(ygroups.pop(0))


            for qq in range((NQ - 1) * NQ, NQ * NQ):
                emit_ygroup(qq, act_copy=(qq % 2 == 0))

    nc.compile()
    return nc


def _shard_inputs(x, context, attn_bias, Wq, Wkv, Wout):
    scale = D ** -0.5
    ident = np.eye(P, dtype=BF16)
    in_maps = []
    for core in range(NCORES):
        b, g = divmod(core, GROUPS)
        cs = slice(g * CG, (g + 1) * CG)
        in_maps.append(
            {
                "xT": np.ascontiguousarray(x[b].T).astype(BF16),
                "cT": np.ascontiguousarray(context[b].T).astype(BF16),
                "biasT": np.ascontiguousarray(
                    attn_bias[b, g * HG : (g + 1) * HG].transpose(0, 2, 1)
                ).astype(BF16),
                "wqT": (Wq[cs, :].T * scale).astype(BF16),
                "wkT": np.ascontiguousarray(Wkv[cs, :].T).astype(BF16),
                "wvT": np.ascontiguousarray(Wkv[E + cs.start : E + cs.stop, :].T).astype(BF16),
                "woT": np.ascontiguousarray(Wout[:, cs].T).astype(BF16),
                "ident": ident,
            }
        )
    return in_maps


def kernel(x, context, attn_bias, Wq, Wkv, Wout, b_out):
    global _CACHED_NC
    if _CACHED_NC is None:
        _CACHED_NC = build_nc()
    nc = _CACHED_NC

    x = np.asarray(x, dtype=np.float32)
    context = np.asarray(context, dtype=np.float32)
    attn_bias = np.asarray(attn_bias, dtype=np.float32)
    Wq = np.asarray(Wq, dtype=np.float32)
    Wkv = np.asarray(Wkv, dtype=np.float32)
    Wout = np.asarray(Wout, dtype=np.float32)
    b_out = np.asarray(b_out, dtype=np.float32)

    in_maps = _shard_inputs(x, context, attn_bias, Wq, Wkv, Wout)
    res = run_bass_kernel_spmd(nc, in_maps, list(range(NCORES)))

    out = np.zeros((B, N, E), dtype=np.float32)
    for core in range(NCORES):
        out[core // GROUPS] += res.results[core]["y"]
    out += b_out.astype(np.float32)
    return out
